# revision 2
# baseline (speedup 1.0000x reference)
"""Trainium2 Bass kernel v2 for nn_MAD_72679436582977 (retrieval_knn).

Block-max scheme (replaces the v1 per-super max8+find_index8 two-pass scan):
  - coarse GEMM K=34 (32 dims | -1*en | +cbias) in f32r, 2-way PE row tiling
    (tile_position (0,0)/(64,0)) so LDWEIGHTS hides under the other group's
    matmul; psum y = 2q.e - |e|^2 - |q|^2 + 2.2 (bias centers fp16 near 0)
  - ACT copies psum -> fp16 SBUF (offloads the DVE); DVE pairwise-max TT tree
    (fp16 2x mode) reduces 16-node blocks -> B[3136] fp32 block maxima
  - global: max8 -> kill rank-1 (self block) via match_replace -> max8 gives
    blocks of ranks 2-9; find_index8 fwd + on a reversed view for fp16-tie
    rescue (duplicate-collapse recovery, up to 2 rescue slabs)
  - indirect-DMA gather of 11 slabs x 16 nodes (self + 8 + 2 rescue), exact
    fp32 rescore nd2 = 2 q.e - (en+qn) (mirrors reference rounding), exact
    top-9, drop self -> 8 neighbors; gather neighbor embeds, logits/softmax
    epilogue as v1.
Data-parallel over edges across 8 cores (128 edges/core, SPMD).
"""
import os
import sys

sys.path.insert(0, "/opt/trn_rl_repo")

import numpy as np

import concourse.bass as bass
import concourse.bacc as bacc
import concourse.mybir as mybir
from concourse import tile
from concourse.bass import IndirectOffsetOnAxis

F32 = mybir.dt.float32
F32R = mybir.dt.float32r
F16 = mybir.dt.float16
U32 = mybir.dt.uint32

N_HEADS = 4
N_NODES = 50000
DIM = 32
N_BATCH = 1024
N_SENT = 8
N_CORES = 8

EDGES_PER_CORE = N_BATCH // N_CORES          # 128
SUPER = 2048
N_SUPERS = 25                                 # 24 full + 1 half (1024)
LAST_W = 1024
N_PAD = SUPER * (N_SUPERS - 1) + LAST_W       # 50176
M_TILES = N_HEADS * 2
KC = DIM + 2                                  # 32 dims + en row + ones row
BLK = 8
NBLOCKS = N_PAD // BLK                        # 3136
SLAB_W = BLK * (DIM + 2)                      # 544 f32 per slab row
N_SLABS = 11                                  # self + 8 + 2 rescue
POOL = N_SLABS * BLK                          # 176
BIAS_C = 2.2
GROUPS = [(0, 4), (4, 4), (8, 4), (12, 4), (16, 4), (20, 4), (24, 1)]

LAST = {}


def _build_program():
    nc = bacc.Bacc(None, num_swdge_queues=4)

    rhs_d = nc.dram_tensor("rhs2", [N_HEADS, 98, N_SUPERS * 1024], F32R,
                           kind="ExternalInput")
    qpack_d = nc.dram_tensor("qpack", [M_TILES, 128, 128], F32R,
                             kind="ExternalInput")
    aux_d = nc.dram_tensor("aux", [M_TILES, 128, 68], F32,
                           kind="ExternalInput")
    embs2_d = nc.dram_tensor("embs2", [N_HEADS * NBLOCKS, SLAB_W], F32,
                             kind="ExternalInput")
    embn_d = nc.dram_tensor("embn", [N_HEADS * N_PAD, DIM], F32,
                            kind="ExternalInput")

    preds_d = nc.dram_tensor("preds", [128, 1], F32, kind="ExternalOutput")
    dbg_gid_d = nc.dram_tensor("dbg_gid", [M_TILES, 128, 8], U32,
                               kind="ExternalOutput")

    with tile.TileContext(nc) as tc:
        with tc.tile_pool(name="const", bufs=1) as cpool, \
             tc.tile_pool(name="qp", bufs=2) as qpool, \
             tc.tile_pool(name="auxp", bufs=2) as auxp, \
             tc.tile_pool(name="rhs", bufs=3) as rpool, \
             tc.tile_pool(name="sf", bufs=2) as sfp, \
             tc.tile_pool(name="tree", bufs=2) as trp, \
             tc.tile_pool(name="bp", bufs=1) as bp, \
             tc.tile_pool(name="gp", bufs=3) as gp, \
             tc.tile_pool(name="prodp", bufs=1) as prodp, \
             tc.tile_pool(name="sm", bufs=3) as sp, \
             tc.tile_pool(name="acc", bufs=1) as accp, \
             tc.tile_pool(name="ps", bufs=2, space="PSUM") as psp:

            # ---- constants ----
            iota8 = cpool.tile([128, 8], F32, tag="iota8")
            nc.gpsimd.iota(iota8[:], pattern=[[1, 8]], base=0,
                           channel_multiplier=0,
                           allow_small_or_imprecise_dtypes=True)
            iotaP = cpool.tile([128, POOL], F32, tag="iotaP")
            nc.gpsimd.iota(iotaP[:], pattern=[[1, POOL]], base=0,
                           channel_multiplier=0,
                           allow_small_or_imprecise_dtypes=True)
            i7a = cpool.tile([128, 7], F32, tag="i7a")   # 1..7
            nc.gpsimd.iota(i7a[:], pattern=[[1, 7]], base=1,
                           channel_multiplier=0,
                           allow_small_or_imprecise_dtypes=True)
            i7d = cpool.tile([128, 7], F32, tag="i7d")   # 7,6,...,1
            nc.vector.tensor_scalar(out=i7d[:], in0=i7a[:], scalar1=-1.0,
                                    scalar2=8.0, op0=mybir.AluOpType.mult,
                                    op1=mybir.AluOpType.add)
            neg_inf8 = cpool.tile([128, 8], F32, tag="neg_inf8")
            nc.vector.memset(neg_inf8[:], -1e30)
            negones8 = cpool.tile([128, 8], F32, tag="negones8")
            nc.vector.memset(negones8[:], -1.0)

            numneg_all = accp.tile([128, M_TILES], F32, tag="numneg")
            wsum_all = accp.tile([128, M_TILES], F32, tag="wsum")

            st = {}

            def phaseA(m):
                """coarse GEMM + ACT fp16 copy + DVE block-max tree -> B."""
                h = m // 2
                qw = qpool.tile([128, 128], F32R, tag="qw")
                nc.sync.dma_start(out=qw[:], in_=qpack_d[m])
                aux = auxp.tile([128, 68], F32, tag="aux")
                nc.sync.dma_start(out=aux[:], in_=aux_d[m])

                B = bp.tile([128, NBLOCKS], F32, tag="B")

                for g0, gn in GROUPS:
                    gw = sum(SUPER if j < N_SUPERS - 1 else LAST_W
                             for j in range(g0, g0 + gn))
                    if gn == 4:
                        sf = sfp.tile([128, 4 * SUPER], F16, tag="sf16")
                    else:
                        sf = sfp.tile([128, LAST_W], F16, tag="sf16L")
                    for ji in range(gn):
                        j = g0 + ji
                        w = SUPER if j < N_SUPERS - 1 else LAST_W
                        rhs_s = rpool.tile([128, 1024], F32R, tag="rhs")
                        nc.sync.dma_start(
                            out=rhs_s[0:98, 0:w // 2],
                            in_=rhs_d[h, :, j * 1024:j * 1024 + w // 2])
                        psum = psp.tile([128, SUPER], F32, tag="ps")
                        for c in range(w // 512):
                            p = c % 2
                            cb = (c // 2) * 512
                            nc.tensor.matmul(
                                psum[:, c * 512:(c + 1) * 512],
                                qw[64 * p:64 * p + KC, :],
                                rhs_s[64 * p:64 * p + KC, cb:cb + 512],
                                start=True, stop=True,
                                tile_position=(64 * p, 0))
                        nc.scalar.activation(
                            sf[:, ji * SUPER:ji * SUPER + w], psum[:, 0:w],
                            mybir.ActivationFunctionType.Copy)
                    # pairwise-max tree: blocks of 16 -> per-block max
                    nb = gw // BLK
                    sfv = sf[:, 0:gw].rearrange("p (b k) -> p b k", k=BLK)
                    t1 = trp.tile([128, 1024, 4], F16, tag="t1")
                    nc.vector.tensor_tensor(
                        out=t1[:, 0:nb], in0=sfv[:, :, 0:4],
                        in1=sfv[:, :, 4:8], op=mybir.AluOpType.max)
                    t2 = trp.tile([128, 1024, 2], F16, tag="t2")
                    nc.vector.tensor_tensor(
                        out=t2[:, 0:nb], in0=t1[:, 0:nb, 0:2],
                        in1=t1[:, 0:nb, 2:4], op=mybir.AluOpType.max)
                    nc.vector.tensor_tensor(
                        out=B[:, g0 * (SUPER // BLK):
                             g0 * (SUPER // BLK) + nb],
                        in0=t2[:, 0:nb, 0], in1=t2[:, 0:nb, 1],
                        op=mybir.AluOpType.max)
                st[m] = {"aux": aux, "B": B}

            def phaseB(m):
                """global top blocks + tie rescue + slab gather issue."""
                h = m // 2
                d = st[m]
                B = d["B"]
                v8 = sp.tile([128, 8], F32, tag="v8")
                nc.vector.max(v8[:], B[:])
                m1x8 = sp.tile([128, 8], F32, tag="m1x8")
                nc.vector.tensor_copy(m1x8[:], neg_inf8[:])
                nc.vector.tensor_copy(m1x8[:, 0:1], v8[:, 0:1])
                srepB = bp.tile([128, NBLOCKS], F32, tag="srepB")
                nc.vector.match_replace(srepB[:], m1x8[:], B[:], -1e30)
                w8b = sp.tile([128, 8], F32, tag="w8b")
                nc.vector.max(w8b[:], srepB[:])
                pf = sp.tile([128, 8], U32, tag="pf")
                nc.vector.max_index(pf[:], w8b[:], srepB[:])
                prr = sp.tile([128, 8], U32, tag="prr")
                nc.vector.max_index(prr[:], w8b[:], srepB[:, ::-1])
                # pr = 3135 - prr  (in fp32)
                prrf = sp.tile([128, 8], F32, tag="prrf")
                nc.vector.tensor_copy(prrf[:], prr[:])
                prf = sp.tile([128, 8], F32, tag="prf")
                nc.vector.scalar_tensor_tensor(
                    out=prf[:], in0=prrf[:], scalar=float(NBLOCKS - 1),
                    in1=negones8[:], op0=mybir.AluOpType.subtract,
                    op1=mybir.AluOpType.mult)
                # duplicate (fp16-tie) detection on adjacent sorted values
                duptie = sp.tile([128, 7], F32, tag="duptie")
                nc.vector.tensor_tensor(out=duptie[:], in0=w8b[:, 1:8],
                                        in1=w8b[:, 0:7],
                                        op=mybir.AluOpType.is_equal)
                dv1 = sp.tile([128, 7], F32, tag="dv1")
                nc.vector.tensor_tensor(out=dv1[:], in0=duptie[:], in1=i7d[:],
                                        op=mybir.AluOpType.mult)
                mx1 = sp.tile([128, 1], F32, tag="mx1")
                nc.vector.tensor_reduce(mx1[:], dv1[:],
                                        axis=mybir.AxisListType.X,
                                        op=mybir.AluOpType.max)
                k1f = sp.tile([128, 1], F32, tag="k1f")   # 7-mx1 (=k1; 7 if none)
                nc.vector.tensor_scalar(out=k1f[:], in0=mx1[:], scalar1=-1.0,
                                        scalar2=7.0, op0=mybir.AluOpType.mult,
                                        op1=mybir.AluOpType.add)
                dv2 = sp.tile([128, 7], F32, tag="dv2")
                nc.vector.tensor_tensor(out=dv2[:], in0=duptie[:], in1=i7a[:],
                                        op=mybir.AluOpType.mult)
                mx2 = sp.tile([128, 1], F32, tag="mx2")
                nc.vector.tensor_reduce(mx2[:], dv2[:],
                                        axis=mybir.AxisListType.X,
                                        op=mybir.AluOpType.max)
                k2f = sp.tile([128, 1], F32, tag="k2f")   # mx2-1 (-1 if none)
                nc.vector.tensor_scalar(out=k2f[:], in0=mx2[:], scalar1=-1.0,
                                        scalar2=None, op0=mybir.AluOpType.add)
                had1 = sp.tile([128, 1], F32, tag="had1")
                nc.vector.tensor_reduce(had1[:], duptie[:],
                                        axis=mybir.AxisListType.X,
                                        op=mybir.AluOpType.max)
                nsum = sp.tile([128, 1], F32, tag="nsum")
                nc.vector.tensor_reduce(nsum[:], duptie[:],
                                        axis=mybir.AxisListType.X,
                                        op=mybir.AluOpType.add)
                had2 = sp.tile([128, 1], F32, tag="had2")
                nc.vector.tensor_scalar(out=had2[:], in0=nsum[:], scalar1=2.0,
                                        scalar2=None, op0=mybir.AluOpType.is_ge)
                # rescue block ids: pr at slot k1 / k2 (0 when none)
                scr8 = sp.tile([128, 8], F32, tag="scr8")
                r1f = sp.tile([128, 1], F32, tag="r1f")
                nc.vector.scalar_tensor_tensor(
                    out=scr8[:], in0=iota8[:, 0:8], scalar=k1f[:],
                    in1=prf[:], op0=mybir.AluOpType.is_equal,
                    op1=mybir.AluOpType.mult, accum_out=r1f[:])
                r2f = sp.tile([128, 1], F32, tag="r2f")
                nc.vector.scalar_tensor_tensor(
                    out=scr8[:], in0=iota8[:, 0:8], scalar=k2f[:],
                    in1=prf[:], op0=mybir.AluOpType.is_equal,
                    op1=mybir.AluOpType.mult, accum_out=r2f[:])
                # guard: if rescue2 == rescue1 (3-way tie), disable rescue2
                req = sp.tile([128, 1], F32, tag="req")
                nc.vector.tensor_scalar(out=req[:], in0=r2f[:],
                                        scalar1=r1f[:], scalar2=None,
                                        op0=mybir.AluOpType.not_equal)
                nc.vector.tensor_tensor(out=had2[:], in0=had2[:], in1=req[:],
                                        op=mybir.AluOpType.mult)
                # slab row table [128, 11] fp32: self | h*NB+pf | h*NB+r1 | +r2
                slabf = sp.tile([128, N_SLABS], F32, tag="slabf")
                nc.vector.tensor_copy(slabf[:, 0:1], d["aux"][:, 66:67])
                pff = sp.tile([128, 8], F32, tag="pff")
                nc.vector.tensor_copy(pff[:], pf[:])
                nc.vector.tensor_scalar(out=slabf[:, 1:9], in0=pff[:],
                                        scalar1=float(h * NBLOCKS),
                                        scalar2=None, op0=mybir.AluOpType.add)
                nc.vector.tensor_scalar(out=slabf[:, 9:10], in0=r1f[:],
                                        scalar1=float(h * NBLOCKS),
                                        scalar2=None, op0=mybir.AluOpType.add)
                nc.vector.tensor_scalar(out=slabf[:, 10:11], in0=r2f[:],
                                        scalar1=float(h * NBLOCKS),
                                        scalar2=None, op0=mybir.AluOpType.add)
                slabu = sp.tile([128, N_SLABS], U32, tag="slabu")
                nc.vector.tensor_copy(slabu[:], slabf[:])
                gath = gp.tile([128, N_SLABS, SLAB_W], F32, tag="gath")
                for s in range(N_SLABS):
                    # dedicated offset tile per gather (mirrors v1's issue
                    # pattern, which spread gathers across ~10 DMA queues)
                    sbu_s = sp.tile([128, 1], U32, tag=f"sbu{s}")
                    nc.vector.tensor_copy(sbu_s[:], slabu[:, s:s + 1])
                    nc.gpsimd.indirect_dma_start(
                        out=gath[:, s], out_offset=None, in_=embs2_d[:],
                        in_offset=IndirectOffsetOnAxis(ap=sbu_s[:], axis=0))
                # rescue-unused masks (per-row -1e30 bias for slots 9/10)
                h1m = sp.tile([128, 1], F32, tag="h1m")
                nc.vector.tensor_scalar(out=h1m[:], in0=had1[:], scalar1=-1.0,
                                        scalar2=None,
                                        op0=mybir.AluOpType.add)
                nc.vector.tensor_scalar(out=h1m[:], in0=h1m[:], scalar1=1e30,
                                        scalar2=None, op0=mybir.AluOpType.mult)
                h2m = sp.tile([128, 1], F32, tag="h2m")
                nc.vector.tensor_scalar(out=h2m[:], in0=had2[:], scalar1=-1.0,
                                        scalar2=None,
                                        op0=mybir.AluOpType.add)
                nc.vector.tensor_scalar(out=h2m[:], in0=h2m[:], scalar1=1e30,
                                        scalar2=None, op0=mybir.AluOpType.mult)
                st[m].update(gath=gath, slabf=slabf, h1m=h1m, h2m=h2m,
                             duptie=duptie)

            def phaseC(m):
                """exact rescore of the 176-node pool + top-9 + epilogue."""
                h = m // 2
                d = st[m]
                aux = d["aux"]
                gath = d["gath"]
                qn_s = aux[:, 0:1]
                qf_s = aux[:, 1:2]
                f_s = aux[:, 2:2 + DIM]
                q_s = aux[:, 34:34 + DIM]

                ev = gath[:].rearrange("p s (k d) -> p s k d", d=DIM + 2)
                prod = prodp.tile([128, N_SLABS, BLK, DIM], F32, tag="prod")
                nc.vector.tensor_tensor(
                    out=prod[:], in0=ev[:, :, :, 0:DIM],
                    in1=q_s.rearrange("p (a b d) -> p a b d", a=1,
                                      b=1).to_broadcast(
                        (128, N_SLABS, BLK, DIM)),
                    op=mybir.AluOpType.mult)
                dot = sp.tile([128, N_SLABS, BLK], F32, tag="dot")
                nc.vector.tensor_reduce(dot[:], prod[:],
                                        axis=mybir.AxisListType.X,
                                        op=mybir.AluOpType.add)
                tt = sp.tile([128, N_SLABS, BLK], F32, tag="tt")
                nc.vector.tensor_scalar(out=tt[:], in0=ev[:, :, :, DIM],
                                        scalar1=qn_s, scalar2=None,
                                        op0=mybir.AluOpType.add)
                nd2 = sp.tile([128, POOL], F32, tag="nd2")
                nc.vector.scalar_tensor_tensor(
                    out=nd2[:], in0=dot[:].rearrange("p s k -> p (s k)"),
                    scalar=2.0, in1=tt[:].rearrange("p s k -> p (s k)"),
                    op0=mybir.AluOpType.mult, op1=mybir.AluOpType.subtract)
                # mask unused rescue slabs
                nc.vector.tensor_scalar(out=nd2[:, 9 * BLK:10 * BLK],
                                        in0=nd2[:, 9 * BLK:10 * BLK],
                                        scalar1=d["h1m"], scalar2=None,
                                        op0=mybir.AluOpType.add)
                nc.vector.tensor_scalar(out=nd2[:, 10 * BLK:11 * BLK],
                                        in0=nd2[:, 10 * BLK:11 * BLK],
                                        scalar1=d["h2m"], scalar2=None,
                                        op0=mybir.AluOpType.add)
                # mask duplicate pf slots (fp16 ties make find_index8 return
                # the same block for adjacent equal values): slab slot 2+j is
                # a dup of slot 1+j iff duptie[j]
                nd2v = nd2[:].rearrange("p (s k) -> p s k", k=BLK)
                nc.vector.scalar_tensor_tensor(
                    out=nd2v[:, 2:9],
                    in0=d["duptie"][:].rearrange(
                        "p (s a) -> p s a", a=1).to_broadcast((128, 7, BLK)),
                    scalar=-1e30, in1=nd2v[:, 2:9],
                    op0=mybir.AluOpType.mult, op1=mybir.AluOpType.add)
                # exact top-9, drop rank-1 (self)
                m1 = sp.tile([128, 1], F32, tag="m1")
                nc.vector.tensor_reduce(m1[:], nd2[:],
                                        axis=mybir.AxisListType.X,
                                        op=mybir.AluOpType.max)
                m1x8b = sp.tile([128, 8], F32, tag="m1x8b")
                nc.vector.tensor_copy(m1x8b[:], neg_inf8[:])
                nc.vector.tensor_copy(m1x8b[:, 0:1], m1[:])
                srepP = sp.tile([128, POOL], F32, tag="srepP")
                nc.vector.match_replace(srepP[:], m1x8b[:], nd2[:], -1e30)
                w8 = sp.tile([128, 8], F32, tag="w8")
                nc.vector.max(w8[:], srepP[:])
                wpos = sp.tile([128, 8], U32, tag="wpos")
                nc.vector.max_index(wpos[:], w8[:], srepP[:])
                # dist/weights: nd2 = -d^2
                w8c = sp.tile([128, 8], F32, tag="w8c")
                nc.vector.tensor_scalar(out=w8c[:], in0=w8[:], scalar1=0.0,
                                        scalar2=None, op0=mybir.AluOpType.min)
                d8 = sp.tile([128, 8], F32, tag="d8")
                nc.scalar.activation(d8[:], w8c[:],
                                     mybir.ActivationFunctionType.Sqrt,
                                     bias=0.0, scale=-1.0)
                wexp8 = sp.tile([128, 8], F32, tag="wexp8")
                nc.scalar.activation(wexp8[:], d8[:],
                                     mybir.ActivationFunctionType.Exp,
                                     bias=1.0, scale=-1.0)
                # winner node gid via pool-position lookup table:
                # gidtab[s*16+k] = slabrow[s]*16 + k  (= h*N_PAD + node)
                wposf = sp.tile([128, 8], F32, tag="wposf")
                nc.vector.tensor_copy(wposf[:], wpos[:])
                gidtab = sp.tile([128, N_SLABS, BLK], F32, tag="gidtab")
                nc.vector.scalar_tensor_tensor(
                    out=gidtab[:],
                    in0=d["slabf"][:].rearrange(
                        "p (s a) -> p s a", a=1).to_broadcast(
                        (128, N_SLABS, BLK)),
                    scalar=float(BLK),
                    in1=iota8[:].rearrange(
                        "p (a k) -> p a k", a=1).to_broadcast(
                        (128, N_SLABS, BLK)),
                    op0=mybir.AluOpType.mult, op1=mybir.AluOpType.add)
                gid_f = sp.tile([128, 8], F32, tag="gid_f")
                scrP = sp.tile([128, POOL], F32, tag="scrP")
                for k in range(8):
                    nc.vector.scalar_tensor_tensor(
                        out=scrP[:], in0=iotaP[:],
                        scalar=wposf[:, k:k + 1],
                        in1=gidtab[:].rearrange("p s k -> p (s k)"),
                        op0=mybir.AluOpType.is_equal,
                        op1=mybir.AluOpType.mult,
                        accum_out=gid_f[:, k:k + 1])
                gid_u = sp.tile([128, 8], U32, tag="gid_u")
                nc.vector.tensor_copy(gid_u[:], gid_f[:])
                nc.sync.dma_start(out=dbg_gid_d[m], in_=gid_u[:])
                # logits u = e.f for the whole pool (avoids a second gather),
                # winners' u extracted by position like gid
                prodf = prodp.tile([128, N_SLABS, BLK, DIM], F32, tag="prodf")
                nc.vector.tensor_tensor(
                    out=prodf[:], in0=ev[:, :, :, 0:DIM],
                    in1=f_s.rearrange("p (a b d) -> p a b d", a=1,
                                      b=1).to_broadcast(
                        (128, N_SLABS, BLK, DIM)),
                    op=mybir.AluOpType.mult)
                upool = sp.tile([128, N_SLABS, BLK], F32, tag="upool")
                nc.vector.tensor_reduce(upool[:], prodf[:],
                                        axis=mybir.AxisListType.X,
                                        op=mybir.AluOpType.add)
                u8 = sp.tile([128, 8], F32, tag="u8")
                for k in range(8):
                    nc.vector.scalar_tensor_tensor(
                        out=scrP[:], in0=iotaP[:],
                        scalar=wposf[:, k:k + 1],
                        in1=upool[:].rearrange("p s k -> p (s k)"),
                        op0=mybir.AluOpType.is_equal,
                        op1=mybir.AluOpType.mult,
                        accum_out=u8[:, k:k + 1])
                scrap8 = sp.tile([128, 8], F32, tag="scrap8")
                nc.vector.scalar_tensor_tensor(
                    out=scrap8[:], in0=u8[:], scalar=qf_s, in1=wexp8[:],
                    op0=mybir.AluOpType.subtract, op1=mybir.AluOpType.mult,
                    accum_out=numneg_all[:, m:m + 1])
                nc.vector.tensor_reduce(wsum_all[:, m:m + 1], wexp8[:],
                                        axis=mybir.AxisListType.X,
                                        op=mybir.AluOpType.add)

            phaseA(0)
            phaseB(0)
            for m in range(1, M_TILES):
                phaseA(m)
                phaseC(m - 1)
                phaseB(m)
            phaseC(M_TILES - 1)

            # combine heads: pred = sigmoid(mean_h num_h / den_h)
            nsum2 = sp.tile([128, N_HEADS], F32, tag="nsum2")
            nc.vector.tensor_reduce(
                nsum2[:], numneg_all[:].rearrange("p (h e) -> p h e", e=2),
                axis=mybir.AxisListType.X, op=mybir.AluOpType.add)
            den = sp.tile([128, N_HEADS], F32, tag="den")
            nc.vector.tensor_reduce(
                den[:], wsum_all[:].rearrange("p (h e) -> p h e", e=2),
                axis=mybir.AxisListType.X, op=mybir.AluOpType.add)
            den8 = sp.tile([128, N_HEADS], F32, tag="den8")
            nc.vector.tensor_scalar(out=den8[:], in0=den[:],
                                    scalar1=float(N_SENT), scalar2=None,
                                    op0=mybir.AluOpType.add)
            rden = sp.tile([128, N_HEADS], F32, tag="rden")
            nc.vector.reciprocal(rden[:], den8[:])
            ratio = sp.tile([128, N_HEADS], F32, tag="ratio")
            nc.vector.tensor_tensor(out=ratio[:], in0=nsum2[:], in1=rden[:],
                                    op=mybir.AluOpType.mult)
            ssum = sp.tile([128, 1], F32, tag="ssum")
            nc.vector.tensor_reduce(ssum[:], ratio[:],
                                    axis=mybir.AxisListType.X,
                                    op=mybir.AluOpType.add)
            preds_s = sp.tile([128, 1], F32, tag="preds")
            nc.scalar.activation(preds_s[:], ssum[:],
                                 mybir.ActivationFunctionType.Sigmoid,
                                 bias=0.0, scale=-1.0 / N_HEADS)
            nc.sync.dma_start(out=preds_d[:], in_=preds_s[:])

    return nc


def _prep_inputs(embeds, field, edges):
    embeds = np.asarray(embeds, dtype=np.float32)
    field = np.asarray(field, dtype=np.float32)
    edges = np.asarray(edges)

    en = np.sum(np.square(embeds), axis=-1, dtype=np.float32)
    en_pad = np.full((N_HEADS, N_PAD), 1000.0, np.float32)
    en_pad[:, :N_NODES] = en

    # Coarse GEMM uses mean-centered coordinates: distances are translation
    # invariant, and centering shrinks the intermediate magnitudes ~10x,
    # cutting f32r cancellation noise in y = 2.2 - d2 accordingly.
    mu = embeds.mean(axis=1, keepdims=True).astype(np.float32)   # (H, 1, D)
    emb_c = embeds - mu
    en_c = np.sum(np.square(emb_c), axis=-1, dtype=np.float32)
    en_c_pad = np.full((N_HEADS, N_PAD), 1000.0, np.float32)
    en_c_pad[:, :N_NODES] = en_c

    # aug [H, KC, N_PAD]: rows 0-31 centered embed dims, 32 centered en, 33 ones
    aug = np.zeros((N_HEADS, KC, N_PAD), dtype=np.float32)
    aug[:, :DIM, :N_NODES] = emb_c.transpose(0, 2, 1)
    aug[:, DIM] = en_c_pad
    aug[:, DIM + 1] = 1.0

    rhs2 = np.zeros((N_HEADS, 98, N_SUPERS * 1024), dtype=np.float32)
    for j in range(N_SUPERS):
        w = SUPER if j < N_SUPERS - 1 else LAST_W
        for c in range(w // 512):
            sl = aug[:, :, j * SUPER + c * 512: j * SUPER + (c + 1) * 512]
            cb = j * 1024 + (c // 2) * 512
            rhs2[:, 64 * (c % 2):64 * (c % 2) + KC, cb:cb + 512] = sl

    # embs2: slab rows [h*NBLOCKS + b] = 16 x (e32 | en_clean | 0)
    emb_pad = np.zeros((N_HEADS, N_PAD, DIM), dtype=np.float32)
    emb_pad[:, :N_NODES] = embeds
    slab = np.zeros((N_HEADS, N_PAD, DIM + 2), dtype=np.float32)
    slab[:, :, :DIM] = emb_pad
    slab[:, :, DIM] = en_pad
    embs2 = slab.reshape(N_HEADS * NBLOCKS, SLAB_W)
    embn = emb_pad.reshape(N_HEADS * N_PAD, DIM)

    in_maps = []
    for core in range(N_CORES):
        sl = slice(core * EDGES_PER_CORE, (core + 1) * EDGES_PER_CORE)
        qpack = np.zeros((M_TILES, 128, 128), dtype=np.float32)
        auxm = np.zeros((M_TILES, 128, 68), dtype=np.float32)
        for m in range(M_TILES):
            h, e = m // 2, m % 2
            nodes = edges[e, sl]
            q = embeds[h, nodes]
            f = field[h, nodes]
            qn = np.einsum('bd,bd->b', q, q)
            q_c = emb_c[h, nodes]
            qn_c = np.einsum('bd,bd->b', q_c, q_c)
            qpack[m, :DIM, :] = (2.0 * q_c).T
            qpack[m, DIM, :] = -1.0
            qpack[m, DIM + 1, :] = BIAS_C - qn_c
            qpack[m, 64:64 + KC, :] = qpack[m, :KC, :]
            auxm[m, :, 0] = qn
            auxm[m, :, 1] = np.einsum('bd,bd->b', q, f)
            auxm[m, :, 2:2 + DIM] = f
            auxm[m, :, 34:34 + DIM] = q
            auxm[m, :, 66] = (h * NBLOCKS + nodes // BLK).astype(np.float32)
        in_maps.append({
            "rhs2": rhs2, "qpack": qpack, "aux": auxm,
            "embs2": embs2, "embn": embn,
        })
    return in_maps


def kernel(embeds, field, edges):
    from concourse.bass_utils import run_bass_kernel_spmd

    nc = _build_program()
    nc.finalize()
    in_maps = _prep_inputs(embeds, field, edges)
    core_ids = list(range(N_CORES))
    trace = bool(os.environ.get("KNN_TRACE"))
    tmpdir = os.environ.get("KNN_TRACE_DIR") or None
    out = run_bass_kernel_spmd(nc, in_maps, core_ids, trace=trace,
                               tmpdir=tmpdir)
    LAST["results"] = out
    preds = np.concatenate(
        [out.results[c]["preds"][:, 0] for c in range(N_CORES)])
    return preds.astype(np.float32)
